# revision 5
# baseline (speedup 1.0000x reference)
"""Multi-head attention (B=2, N=2048, C=512, H=8) on 8 trn2 NeuronCores.

Sharding: tensor-parallel over heads x data-parallel over batch.
Core i handles batch b = i//4 and heads {2*(i%4), 2*(i%4)+1} (a contiguous
128-column slice of Wq/Wk/Wv and 128-row slice of Wo). Each core computes
its heads' full attention and a partial output projection; the host sums
the 4 partials per batch (the TP row-split unshard) and stacks batches.

v3 design (vs ~200us fp32r v1, ~190us f16 v2):
  - Host pre-transposes q/kv/pos to channel-major and pre-casts all matmul
    operands to fp16 (no on-device input transposes, half the DMA bytes).
  - fp16 matmul operands everywhere, fp32 PSUM accumulation.
  - exp is split across TWO engines per k-chunk: by (kc+h) parity one
    head's [128,1024] S tile goes through ACT's table EXP (~1.11us) and
    the other through a DVE Schraudolph exp (~1.22us): one tensor_scalar
    computing int16(s*SCALE*2^10/ln2 + (15<<10) - magic), whose bit
    pattern IS fp16 exp to 4% max element error; bitcast to f16 feeds PV.
    Softmax renormalization (ones-column denominators summed from the
    same approximate values) cancels most of it: measured end-to-end
    max-rel error ~0.9e-2 in numpy simulation incl. the f16 pipeline.
  - PSUM: 2 single-buffered S tiles [128,1024] + 2 PV accumulators
    [128,1024] = 8 banks. exp of head h chases S bank-pair by bank-pair.
  - PV lhsT is padded to 128 columns ([d(64)|ones|pad] stride-72 layout)
    to enable the fast-weight-load path; the garbage output rows 65-127
    of the PV accumulator are never read.
  - PV issue lags S/exp by 3 k-chunks so the PE queue never blocks on the
    q-half boundary (PV(qh2=1,kc0) waits for the previous half's
    normalize to release the accumulator banks).
  - Input DMAs ride the two HWDGE queues interleaved with the weight/pos
    loads (v2 put 1.6MB on the slow SWDGE path, gating phase 1).
"""
import numpy as np

B, N, C, H, D = 2, 2048, 512, 8, 64
SCALE = float(C) ** -0.5
NCORES = 8
P = 128
CC = C // P            # 4 channel chunks of 128
NT16 = N // P          # 16 token tiles of 128
NT4 = N // 512         # 4 token slabs of 512
PVLAG = 3              # PV issue lag in k-chunks

# Schraudolph fp16 exp constants (folding the attention scale):
#   i16 = rni(s * SCH_A + SCH_B); bitcast f16 ~= exp(s * SCALE)
SCH_A = SCALE * (2.0 ** 10) / float(np.log(2.0))
SCH_B = 15.0 * 1024.0 - 486411.0 / 8192.0

_cached_nc = None


def _build():
    from contextlib import ExitStack

    import concourse.mybir as mybir
    import concourse.tile as tile
    from concourse import bacc
    from concourse.alu_op_type import AluOpType
    from concourse.tile_rust import add_dep_helper

    f32 = mybir.dt.float32
    f16 = mybir.dt.float16
    i16 = mybir.dt.int16
    AF = mybir.ActivationFunctionType

    nc = bacc.Bacc("TRN2", target_bir_lowering=False, debug=False)

    # host-prepared, channel-major, fp16
    qT = nc.dram_tensor("qT", [P, CC, N], f16, kind="ExternalInput")
    kvT = nc.dram_tensor("kvT", [P, CC, N], f16, kind="ExternalInput")
    posq = nc.dram_tensor("posq", [P, N], f16, kind="ExternalInput")  # 2x stacked
    posk = nc.dram_tensor("posk", [P, N], f16, kind="ExternalInput")
    wq = nc.dram_tensor("wq", [P, CC, P], f16, kind="ExternalInput")
    wk = nc.dram_tensor("wk", [P, CC, P], f16, kind="ExternalInput")
    wv = nc.dram_tensor("wv", [P, CC, P], f16, kind="ExternalInput")
    wo = nc.dram_tensor("wo", [P, C], f16, kind="ExternalInput")
    bo4 = nc.dram_tensor("bo4", [1, C], f32, kind="ExternalInput")
    eye = nc.dram_tensor("eye", [P, P], f16, kind="ExternalInput")
    y = nc.dram_tensor("y", [N, C], f32, kind="ExternalOutput")

    with tile.TileContext(nc) as tc, ExitStack() as ctx:
        persist = ctx.enter_context(tc.tile_pool(name="persist", bufs=1))

        # ---- input DMAs on the two HWDGE queues, weights-first ----
        qT_sb = persist.tile([P, CC, N], f16, tag="qT_sb")
        kvT_sb = persist.tile([P, CC, N], f16, tag="kvT_sb")
        w_sb = {name: persist.tile([P, CC, P], f16, tag=f"{name}_sb",
                                   name=f"{name}_sb")
                for name in ("wq", "wk", "wv")}
        pos_sb = {name: persist.tile([P, N], f16, tag=f"pos_{name}",
                                     name=f"pos_{name}")
                  for name in ("q", "k")}
        eye_sb = persist.tile([P, P], f16, tag="eye_sb")

        nc.sync.dma_start(w_sb["wq"][:], wq[:])
        nc.scalar.dma_start(w_sb["wk"][:], wk[:])
        nc.scalar.dma_start(w_sb["wv"][:], wv[:])
        nc.scalar.dma_start(eye_sb[:], eye[:])
        for ntt in range(NT4):
            sl = slice(ntt * 512, (ntt + 1) * 512)
            nc.sync.dma_start(qT_sb[:, :, sl], qT[:, :, sl])
            nc.scalar.dma_start(kvT_sb[:, :, sl], kvT[:, :, sl])
            if ntt == 0:
                nc.sync.dma_start(pos_sb["q"][:], posq[:])
                nc.scalar.dma_start(pos_sb["k"][:], posk[:])
        # phase-3-only loads on the swdge path
        wo_r = []
        for h in (0, 1):
            wr = persist.tile([64, C], f16, tag=f"wo_r{h}")
            nc.gpsimd.dma_start(wr[:], wo[64 * h:64 * (h + 1), :])
            wo_r.append(wr)
        bo_bc = persist.tile([P, C], f32, tag="bo_bc")
        nc.gpsimd.dma_start(bo_bc[:], bo4[:].to_broadcast((P, C)))

        # PE order pinned with order-only deps: the PE queue is in-order,
        # so this fixes the matmul stream layout.
        pe_prev = [None]

        def chain(mm):
            if pe_prev[0] is not None:
                add_dep_helper(mm.ins, pe_prev[0].ins, sync=False,
                               reason="pin PE order")
            pe_prev[0] = mm

        # ---- phase 1: projections (channel-major out), vp token-major ----
        qhT = persist.tile([P, N], f16, tag="qhT")
        khT = persist.tile([P, N], f16, tag="khT")
        # vp: flat padded layout, per kc at offset 144*kc:
        #   [h0 d0..63 | ones | pad7 | h1 d0..63 | ones | pad7]
        # lhsT(kc,h) = vp[:, 144*kc+72*h : +128] -> 128 cols (FWL-eligible),
        # cols past 65 are finite garbage feeding unread PV output rows.
        vp = persist.tile([P, NT16 * 144 + 56], f16, tag="vp")
        vp4 = vp[:, 0:NT16 * 144].rearrange("p (a b c) -> p a b c", a=NT16, b=2)
        fill_col = persist.tile([P, 1], f16, tag="fill_col")
        nc.gpsimd.memset(fill_col[:], 0.0)
        nc.vector.tensor_copy(vp4[:, :, :, 65:72],
                              fill_col[:].to_broadcast((P, NT16, 2, 7)))
        nc.vector.tensor_copy(vp[:, NT16 * 144:],
                              fill_col[:].to_broadcast((P, 56)))
        ones_col = persist.tile([P, 1], f16, tag="ones_col")
        nc.gpsimd.memset(ones_col[:], 1.0)
        nc.vector.tensor_copy(vp4[:, :, :, 64:65],
                              ones_col[:].to_broadcast((P, NT16, 2, 1)))

        with (
            tc.tile_pool(name="proj_ps", bufs=3, space="PSUM") as proj_ps,
            tc.tile_pool(name="tp_ps", bufs=4, space="PSUM") as tp_ps,
            tc.tile_pool(name="big", bufs=1) as big,
        ):
            vpT = big.tile([P, N], f16, tag="vpT")
            for ntt in range(NT4):
                sl = slice(ntt * 512, (ntt + 1) * 512)
                for wname, srcT, dstT, posn in (
                    ("wq", qT_sb, qhT, "q"),
                    ("wk", kvT_sb, khT, "k"),
                ):
                    pp = proj_ps.tile([P, 512], f32, tag="proj")
                    for cc in range(CC):
                        chain(nc.tensor.matmul(
                            pp[:], w_sb[wname][:, cc], srcT[:, cc, sl],
                            start=(cc == 0), stop=(cc == CC - 1)))
                    nc.vector.tensor_add(
                        out=dstT[:, sl], in0=pp[:], in1=pos_sb[posn][:, sl])
                pp = proj_ps.tile([P, 512], f32, tag="proj")
                for cc in range(CC):
                    chain(nc.tensor.matmul(
                        pp[:], w_sb["wv"][:, cc], kvT_sb[:, cc, sl],
                        start=(cc == 0), stop=(cc == CC - 1)))
                nc.scalar.copy(vpT[:, sl], pp[:])
                # transpose this slab's 4 token tiles into vp
                for t in range(ntt * 4, ntt * 4 + 4):
                    tp = tp_ps.tile([P, P], f16, tag="tp")
                    chain(nc.tensor.matmul(
                        tp[:], vpT[:, t * P:(t + 1) * P], eye_sb[:],
                        is_transpose=True))
                    nc.vector.tensor_copy(
                        vp4[:, t, :, 0:64],
                        tp[:].rearrange("p (h d) -> p h d", h=2))

        # ---- phase 2: attention ----
        O_sb = persist.tile([64, 2, N], f16, tag="O_sb")
        with (
            tc.tile_pool(name="st_ps", bufs=1, space="PSUM") as st_ps,
            tc.tile_pool(name="ot_ps", bufs=1, space="PSUM") as ot_ps,
            tc.tile_pool(name="expp", bufs=8) as expp,
            tc.tile_pool(name="den", bufs=2) as den_pool,
            tc.tile_pool(name="dend", bufs=2, space="DRAM") as den_dram_pool,
        ):
            for qh2 in range(2):
                q_sl = slice(qh2 * 1024, (qh2 + 1) * 1024)
                OT = [ot_ps.tile([P, 1024], f32, tag=f"ot{h}",
                                 name=f"ot{h}") for h in (0, 1)]

                def s_pair(kc, h):
                    st = st_ps.tile([P, 1024], f32, tag=f"st{h}",
                                    name=f"st{h}")
                    hsl = slice(64 * h, 64 * h + 64)
                    for qq in range(2):
                        chain(nc.tensor.matmul(
                            st[:, qq * 512:(qq + 1) * 512],
                            khT[hsl, kc * P:(kc + 1) * P],
                            qhT[hsl, (qh2 * 2 + qq) * 512:
                                     (qh2 * 2 + qq + 1) * 512],
                            start=True, stop=True))
                    ex = expp.tile([P, 1024], f16, tag=f"ex{h}",
                                   name=f"ex{h}")
                    if (kc + h) % 2 == 0:
                        nc.scalar.activation(ex[:], st[:], AF.Exp,
                                             scale=SCALE)
                    else:
                        nc.vector.tensor_scalar(
                            ex[:].bitcast(i16), st[:], SCH_A, SCH_B,
                            AluOpType.mult, AluOpType.add)
                    return ex

                def pv_quad(kc, exs):
                    for h in (0, 1):
                        lhsT = vp[:, 144 * kc + 72 * h:144 * kc + 72 * h + 128]
                        for qq in range(2):
                            chain(nc.tensor.matmul(
                                OT[h][:, qq * 512:(qq + 1) * 512],
                                lhsT,
                                exs[h][:, qq * 512:(qq + 1) * 512],
                                start=(kc == 0), stop=(kc == NT16 - 1)))

                exq = []
                for kc in range(NT16):
                    exq.append([s_pair(kc, 0), s_pair(kc, 1)])
                    if kc >= PVLAG:
                        pv_quad(kc - PVLAG, exq[kc - PVLAG])
                for kc in range(NT16 - PVLAG, NT16):
                    pv_quad(kc, exq[kc])

                # normalize: O = O' / den; den-row copies on ACT, recip +
                # multiplies on DVE, broadcast via DRAM bounce.
                den_r = den_pool.tile([P, 2, 1024], f32, tag="den_r")
                for h in (0, 1):
                    nc.scalar.copy(den_r[64:65, h, :], OT[h][64:65, :])
                den_d = den_dram_pool.tile([1, 2, 1024], f32, tag="den_d")
                nc.sync.dma_start(den_d[:], den_r[64:65, :, :])
                den_bc = den_pool.tile([64, 2, 1024], f32, tag="den_bc")
                nc.sync.dma_start(
                    den_bc[:], den_d[:].to_broadcast((64, 2, 1024)))
                den_rec = den_pool.tile([64, 2, 1024], f32, tag="den_rec")
                nc.vector.reciprocal_approx_fast(den_rec[:], den_bc[:])
                for h in (0, 1):
                    nc.vector.tensor_mul(
                        out=O_sb[:, h, q_sl], in0=OT[h][0:64, :],
                        in1=den_rec[:, h, :])

        # ---- phase 3: output projection, token-major ----
        with (
            tc.tile_pool(name="y_ps", bufs=2, space="PSUM") as y_ps,
            tc.tile_pool(name="yout", bufs=3) as yout,
        ):
            for t in range(NT16):
                tsl = slice(t * P, (t + 1) * P)
                yp = y_ps.tile([P, C], f32, tag="y")
                for h in (0, 1):
                    chain(nc.tensor.matmul(
                        yp[:], O_sb[:, h, tsl], wo_r[h][:],
                        start=(h == 0), stop=(h == 1)))
                ysb = yout.tile([P, C], f32, tag="ysb")
                nc.vector.tensor_add(out=ysb[:], in0=yp[:], in1=bo_bc[:])
                oeng = nc.sync if t % 2 == 0 else nc.scalar
                oeng.dma_start(y[tsl, :], ysb[:])

    nc.finalize()
    return nc


def _chmajor(x):
    # [N, C] token-major f32 -> [P, CC, N] channel-major chunked f16
    return np.ascontiguousarray(
        x.T.reshape(CC, P, N).transpose(1, 0, 2)).astype(np.float16)


def _wchunk(w):
    # [C, P] -> [P, CC, P] lhsT chunks
    return np.ascontiguousarray(
        w.reshape(CC, P, P).transpose(1, 0, 2)).astype(np.float16)


def _in_maps(q, kv, pos_q, pos_k, Wq, Wk, Wv, Wo, bo):
    maps = []
    for i in range(NCORES):
        b, hp = i // 4, i % 4
        cs = P * hp
        posqT = pos_q[b].T.astype(np.float16)   # [64, N]
        poskT = pos_k[b].T.astype(np.float16)
        maps.append({
            "qT": _chmajor(np.asarray(q[b], dtype=np.float32)),
            "kvT": _chmajor(np.asarray(kv[b], dtype=np.float32)),
            "posq": np.ascontiguousarray(np.concatenate([posqT, posqT], 0)),
            "posk": np.ascontiguousarray(np.concatenate([poskT, poskT], 0)),
            "wq": _wchunk(np.asarray(Wq[:, cs:cs + P], dtype=np.float32)),
            "wk": _wchunk(np.asarray(Wk[:, cs:cs + P], dtype=np.float32)),
            "wv": _wchunk(np.asarray(Wv[:, cs:cs + P], dtype=np.float32)),
            "wo": np.ascontiguousarray(Wo[cs:cs + P, :]).astype(np.float16),
            "bo4": (np.asarray(bo, dtype=np.float32) / 4.0)[None, :],
            "eye": np.eye(P, dtype=np.float16),
        })
    return maps


def kernel(q, kv, pos_q, pos_k, Wq, Wk, Wv, Wo, bo):
    from concourse.bass_utils import run_bass_kernel_spmd

    global _cached_nc
    if _cached_nc is None:
        _cached_nc = _build()

    args = [np.asarray(a) for a in (q, kv, pos_q, pos_k, Wq, Wk, Wv, Wo, bo)]
    maps = _in_maps(*args)
    res = run_bass_kernel_spmd(_cached_nc, maps, list(range(NCORES)))
    outs = [res.results[i]["y"] for i in range(NCORES)]
    y0 = outs[0] + outs[1] + outs[2] + outs[3]
    y1 = outs[4] + outs[5] + outs[6] + outs[7]
    return np.stack([y0, y1]).astype(np.float32)


# revision 11
# speedup vs baseline: 1.0208x; 1.0208x over previous
"""Multi-head attention (B=2, N=2048, C=512, H=8) on 8 trn2 NeuronCores.

Sharding: tensor-parallel over heads x data-parallel over batch.
Core i handles batch b = i//4 and heads {2*(i%4), 2*(i%4)+1} (a contiguous
128-column slice of Wq/Wk/Wv and 128-row slice of Wo). Each core computes
its heads' full attention and a partial output projection; the host sums
the 4 partials per batch (the TP row-split unshard) and stacks batches.

v3 design (vs ~200us fp32r v1, ~190us f16 v2):
  - Host pre-transposes q/kv/pos to channel-major and pre-casts all matmul
    operands to fp16 (no on-device input transposes, half the DMA bytes).
  - fp16 matmul operands everywhere, fp32 PSUM accumulation.
  - exp is split across TWO engines per k-chunk: by (kc+h) parity one
    head's [128,1024] S tile goes through ACT's table EXP (~1.11us) and
    the other through a DVE Schraudolph exp (~1.22us): one tensor_scalar
    computing int16(s*SCALE*2^10/ln2 + (15<<10) - magic), whose bit
    pattern IS fp16 exp to 4% max element error; bitcast to f16 feeds PV.
    Softmax renormalization (ones-column denominators summed from the
    same approximate values) cancels most of it: measured end-to-end
    max-rel error ~0.9e-2 in numpy simulation incl. the f16 pipeline.
  - PSUM: 2 single-buffered S tiles [128,1024] + 2 PV accumulators
    [128,1024] = 8 banks. exp of head h chases S bank-pair by bank-pair.
  - PV lhsT is padded to 128 columns ([d(64)|ones|pad] stride-72 layout)
    to enable the fast-weight-load path; the garbage output rows 65-127
    of the PV accumulator are never read.
  - PV issue lags S/exp by 3 k-chunks so the PE queue never blocks on the
    q-half boundary (PV(qh2=1,kc0) waits for the previous half's
    normalize to release the accumulator banks).
  - Input DMAs ride the two HWDGE queues interleaved with the weight/pos
    loads (v2 put 1.6MB on the slow SWDGE path, gating phase 1).
"""
import numpy as np

B, N, C, H, D = 2, 2048, 512, 8, 64
SCALE = float(C) ** -0.5
NCORES = 8
P = 128
CC = C // P            # 4 channel chunks of 128
NT16 = N // P          # 16 token tiles of 128
NT4 = N // 512         # 4 token slabs of 512
PVLAG = 3              # PV issue lag in k-chunks

# Schraudolph fp16 exp constants (folding the attention scale):
#   i16 = rni(s * SCH_A + SCH_B); bitcast f16 ~= exp(s * SCALE)
SCH_A = SCALE * (2.0 ** 10) / float(np.log(2.0))
SCH_B = 15.0 * 1024.0 - 486411.0 / 8192.0

_cached_nc = None


def _build():
    from contextlib import ExitStack

    import concourse.mybir as mybir
    import concourse.tile as tile
    from concourse import bacc
    from concourse.alu_op_type import AluOpType
    from concourse.tile_rust import add_dep_helper

    f32 = mybir.dt.float32
    f16 = mybir.dt.float16
    i16 = mybir.dt.int16
    AF = mybir.ActivationFunctionType

    nc = bacc.Bacc("TRN2", target_bir_lowering=False, debug=False)

    # host-prepared, channel-major, fp16
    qT = nc.dram_tensor("qT", [P, CC, N], f16, kind="ExternalInput")
    kvT = nc.dram_tensor("kvT", [P, CC, N], f16, kind="ExternalInput")
    posq = nc.dram_tensor("posq", [P, N], f16, kind="ExternalInput")  # 2x stacked
    posk = nc.dram_tensor("posk", [P, N], f16, kind="ExternalInput")
    wq = nc.dram_tensor("wq", [P, CC, P], f16, kind="ExternalInput")
    wk = nc.dram_tensor("wk", [P, CC, P], f16, kind="ExternalInput")
    wv = nc.dram_tensor("wv", [P, CC, P], f16, kind="ExternalInput")
    wo = nc.dram_tensor("wo", [P, C], f16, kind="ExternalInput")
    bo4f = nc.dram_tensor("bo4", [1, C], f16, kind="ExternalInput")
    eye = nc.dram_tensor("eye", [P, P], f16, kind="ExternalInput")
    y = nc.dram_tensor("y", [N, C], f32, kind="ExternalOutput")

    with tile.TileContext(nc) as tc, ExitStack() as ctx:
        persist = ctx.enter_context(tc.tile_pool(name="persist", bufs=1))

        # ---- input DMAs on the two HWDGE queues, weights-first ----
        qT_sb = persist.tile([P, CC, N], f16, tag="qT_sb")
        kvT_sb = persist.tile([P, CC, N], f16, tag="kvT_sb")
        w_sb = {name: persist.tile([P, CC, P], f16, tag=f"{name}_sb",
                                   name=f"{name}_sb")
                for name in ("wq", "wk", "wv")}
        pos_sb = {name: persist.tile([P, N], f16, tag=f"pos_{name}",
                                     name=f"pos_{name}")
                  for name in ("q", "k")}
        eye_sb = persist.tile([P, P], f16, tag="eye_sb")

        nc.sync.dma_start(w_sb["wq"][:], wq[:])
        nc.scalar.dma_start(w_sb["wk"][:], wk[:])
        nc.scalar.dma_start(w_sb["wv"][:], wv[:])
        nc.scalar.dma_start(eye_sb[:], eye[:])
        for ntt in range(NT4):
            sl = slice(ntt * 512, (ntt + 1) * 512)
            nc.sync.dma_start(qT_sb[:, :, sl], qT[:, :, sl])
            nc.scalar.dma_start(kvT_sb[:, :, sl], kvT[:, :, sl])
            if ntt == 0:
                nc.sync.dma_start(pos_sb["q"][:], posq[:])
                nc.scalar.dma_start(pos_sb["k"][:], posk[:])
        # phase-3-only loads on the swdge path
        wo_r = []
        for h in (0, 1):
            wr = persist.tile([64, C], f16, tag=f"wo_r{h}")
            nc.gpsimd.dma_start(wr[:], wo[64 * h:64 * (h + 1), :])
            wo_r.append(wr)
        bo_sb = persist.tile([1, C], f16, tag="bo_sb")
        nc.gpsimd.dma_start(bo_sb[:], bo4f[:])
        ones_row = persist.tile([1, P], f16, tag="ones_row")
        nc.gpsimd.memset(ones_row[:], 1.0)

        # PE order pinned with order-only deps: the PE queue is in-order,
        # so this fixes the matmul stream layout.
        pe_prev = [None]

        def chain(mm):
            if pe_prev[0] is not None:
                add_dep_helper(mm.ins, pe_prev[0].ins, sync=False,
                               reason="pin PE order")
            pe_prev[0] = mm

        # ---- phase 1: projections (channel-major out), vp token-major ----
        qhT = persist.tile([P, N], f16, tag="qhT")
        khT = persist.tile([P, N], f16, tag="khT")
        # vp: flat padded layout, per kc at offset 144*kc:
        #   [h0 d0..63 | ones | pad7 | h1 d0..63 | ones | pad7]
        # lhsT(kc,h) = vp[:, 144*kc+72*h : +128] -> 128 cols (FWL-eligible),
        # cols past 65 are finite garbage feeding unread PV output rows.
        vp = persist.tile([P, NT16 * 144 + 56], f16, tag="vp")
        vp4 = vp[:, 0:NT16 * 144].rearrange("p (a b c) -> p a b c", a=NT16, b=2)
        fill_col = persist.tile([P, 1], f16, tag="fill_col")
        nc.gpsimd.memset(fill_col[:], 0.0)
        nc.vector.tensor_copy(vp4[:, :, :, 65:72],
                              fill_col[:].to_broadcast((P, NT16, 2, 7)))
        nc.vector.tensor_copy(vp[:, NT16 * 144:],
                              fill_col[:].to_broadcast((P, 56)))
        ones_col = persist.tile([P, 1], f16, tag="ones_col")
        nc.gpsimd.memset(ones_col[:], 1.0)
        nc.vector.tensor_copy(vp4[:, :, :, 64:65],
                              ones_col[:].to_broadcast((P, NT16, 2, 1)))

        with (
            tc.tile_pool(name="proj_ps", bufs=3, space="PSUM") as proj_ps,
            tc.tile_pool(name="tp_ps", bufs=4, space="PSUM") as tp_ps,
            tc.tile_pool(name="big", bufs=1) as big,
        ):
            vpT = big.tile([P, N], f16, tag="vpT")
            for ntt in range(NT4):
                sl = slice(ntt * 512, (ntt + 1) * 512)
                for wname, srcT, dstT, posn in (
                    ("wq", qT_sb, qhT, "q"),
                    ("wk", kvT_sb, khT, "k"),
                ):
                    pp = proj_ps.tile([P, 512], f32, tag="proj")
                    for cc in range(CC):
                        chain(nc.tensor.matmul(
                            pp[:], w_sb[wname][:, cc], srcT[:, cc, sl],
                            start=(cc == 0), stop=(cc == CC - 1)))
                    nc.vector.tensor_add(
                        out=dstT[:, sl], in0=pp[:], in1=pos_sb[posn][:, sl])
                pp = proj_ps.tile([P, 512], f32, tag="proj")
                for cc in range(CC):
                    chain(nc.tensor.matmul(
                        pp[:], w_sb["wv"][:, cc], kvT_sb[:, cc, sl],
                        start=(cc == 0), stop=(cc == CC - 1)))
                nc.scalar.copy(vpT[:, sl], pp[:])
                # transpose this slab's 4 token tiles into vp
                for t in range(ntt * 4, ntt * 4 + 4):
                    tp = tp_ps.tile([P, P], f16, tag="tp")
                    chain(nc.tensor.matmul(
                        tp[:], vpT[:, t * P:(t + 1) * P], eye_sb[:],
                        is_transpose=True))
                    nc.vector.tensor_copy(
                        vp4[:, t, :, 0:64],
                        tp[:].rearrange("p (h d) -> p h d", h=2))

        # ---- phase 2: attention ----
        O_sb = persist.tile([64, 2, N], f16, tag="O_sb")
        with (
            tc.tile_pool(name="st_ps", bufs=2, space="PSUM") as st_ps,
            tc.tile_pool(name="ot_ps", bufs=1, space="PSUM") as ot_ps,
            tc.tile_pool(name="expp", bufs=8) as expp,
            tc.tile_pool(name="den", bufs=2) as den_pool,
            tc.tile_pool(name="dend", bufs=2, space="DRAM") as den_dram_pool,
        ):
            for qh2 in range(2):
                q_sl = slice(qh2 * 1024, (qh2 + 1) * 1024)
                OT = [ot_ps.tile([P, 1024], f32, tag=f"ot{h}",
                                 name=f"ot{h}") for h in (0, 1)]

                def s_pair(kc, h):
                    # two [128,512] S tiles (1 PSUM bank each, bufs=2 per
                    # head-lineage) so next kc's S never waits on this exp
                    ex = expp.tile([P, 1024], f16, tag=f"ex{h}",
                                   name=f"ex{h}")
                    hsl = slice(64 * h, 64 * h + 64)
                    for qq in range(2):
                        st = st_ps.tile([P, 512], f32, tag=f"st{h}",
                                        name=f"st{h}q{qq}")
                        chain(nc.tensor.matmul(
                            st[:],
                            khT[hsl, kc * P:(kc + 1) * P],
                            qhT[hsl, (qh2 * 2 + qq) * 512:
                                     (qh2 * 2 + qq + 1) * 512],
                            start=True, stop=True))
                        exq_sl = ex[:, qq * 512:(qq + 1) * 512]
                        if (kc + h) % 2 == 0:
                            nc.scalar.activation(exq_sl, st[:], AF.Exp,
                                                 scale=SCALE)
                        else:
                            nc.vector.tensor_scalar(
                                exq_sl.bitcast(i16), st[:], SCH_A, SCH_B,
                                AluOpType.mult, AluOpType.add)
                    return ex

                def pv_quad(kc, exs):
                    for h in (0, 1):
                        lhsT = vp[:, 144 * kc + 72 * h:144 * kc + 72 * h + 128]
                        for qq in range(2):
                            chain(nc.tensor.matmul(
                                OT[h][:, qq * 512:(qq + 1) * 512],
                                lhsT,
                                exs[h][:, qq * 512:(qq + 1) * 512],
                                start=(kc == 0), stop=(kc == NT16 - 1)))

                exq = []
                for kc in range(NT16):
                    exq.append([s_pair(kc, 0), s_pair(kc, 1)])
                    if kc >= PVLAG:
                        pv_quad(kc - PVLAG, exq[kc - PVLAG])
                for kc in range(NT16 - PVLAG, NT16):
                    pv_quad(kc, exq[kc])

                # normalize: O = O' / den; den-row copies on ACT, recip +
                # multiplies on DVE, broadcast via DRAM bounce.
                den_r = den_pool.tile([P, 2, 1024], f32, tag="den_r")
                for h in (0, 1):
                    nc.scalar.copy(den_r[64:65, h, :], OT[h][64:65, :])
                den_d = den_dram_pool.tile([1, 2, 1024], f32, tag="den_d")
                nc.sync.dma_start(den_d[:], den_r[64:65, :, :])
                den_bc = den_pool.tile([64, 2, 1024], f32, tag="den_bc")
                nc.sync.dma_start(
                    den_bc[:], den_d[:].to_broadcast((64, 2, 1024)))
                den_rec = den_pool.tile([64, 2, 1024], f32, tag="den_rec")
                nc.vector.reciprocal_approx_fast(den_rec[:], den_bc[:])
                for h in (0, 1):
                    nc.vector.tensor_mul(
                        out=O_sb[:, h, q_sl], in0=OT[h][0:64, :],
                        in1=den_rec[:, h, :])

        # ---- phase 3: output projection, token-major ----
        with (
            tc.tile_pool(name="y_ps", bufs=2, space="PSUM") as y_ps,
            tc.tile_pool(name="yout", bufs=3) as yout,
        ):
            for t in range(NT16):
                tsl = slice(t * P, (t + 1) * P)
                yp = y_ps.tile([P, C], f32, tag="y")
                # bias via K=1 ones-row matmul so evacuation is a pure copy
                chain(nc.tensor.matmul(yp[:], ones_row[:], bo_sb[:],
                                       start=True, stop=False))
                for h in (0, 1):
                    chain(nc.tensor.matmul(
                        yp[:], O_sb[:, h, tsl], wo_r[h][:],
                        start=False, stop=(h == 1)))
                ysb = yout.tile([P, C], f32, tag="ysb")
                if t % 2 == 0:
                    nc.scalar.copy(ysb[:], yp[:])
                    nc.sync.dma_start(y[tsl, :], ysb[:])
                else:
                    nc.vector.tensor_copy(ysb[:], yp[:])
                    nc.scalar.dma_start(y[tsl, :], ysb[:])

    nc.finalize()
    return nc


def _chmajor(x):
    # [N, C] token-major f32 -> [P, CC, N] channel-major chunked f16
    return np.ascontiguousarray(
        x.T.reshape(CC, P, N).transpose(1, 0, 2)).astype(np.float16)


def _wchunk(w):
    # [C, P] -> [P, CC, P] lhsT chunks
    return np.ascontiguousarray(
        w.reshape(CC, P, P).transpose(1, 0, 2)).astype(np.float16)


def _in_maps(q, kv, pos_q, pos_k, Wq, Wk, Wv, Wo, bo):
    maps = []
    for i in range(NCORES):
        b, hp = i // 4, i % 4
        cs = P * hp
        posqT = pos_q[b].T.astype(np.float16)   # [64, N]
        poskT = pos_k[b].T.astype(np.float16)
        maps.append({
            "qT": _chmajor(np.asarray(q[b], dtype=np.float32)),
            "kvT": _chmajor(np.asarray(kv[b], dtype=np.float32)),
            "posq": np.ascontiguousarray(np.concatenate([posqT, posqT], 0)),
            "posk": np.ascontiguousarray(np.concatenate([poskT, poskT], 0)),
            "wq": _wchunk(np.asarray(Wq[:, cs:cs + P], dtype=np.float32)),
            "wk": _wchunk(np.asarray(Wk[:, cs:cs + P], dtype=np.float32)),
            "wv": _wchunk(np.asarray(Wv[:, cs:cs + P], dtype=np.float32)),
            "wo": np.ascontiguousarray(Wo[cs:cs + P, :]).astype(np.float16),
            "bo4": (np.asarray(bo, dtype=np.float32) / 4.0
                    ).astype(np.float16)[None, :],
            "eye": np.eye(P, dtype=np.float16),
        })
    return maps


def kernel(q, kv, pos_q, pos_k, Wq, Wk, Wv, Wo, bo):
    from concourse.bass_utils import run_bass_kernel_spmd

    global _cached_nc
    if _cached_nc is None:
        _cached_nc = _build()

    args = [np.asarray(a) for a in (q, kv, pos_q, pos_k, Wq, Wk, Wv, Wo, bo)]
    maps = _in_maps(*args)
    res = run_bass_kernel_spmd(_cached_nc, maps, list(range(NCORES)))
    outs = [res.results[i]["y"] for i in range(NCORES)]
    y0 = outs[0] + outs[1] + outs[2] + outs[3]
    y1 = outs[4] + outs[5] + outs[6] + outs[7]
    return np.stack([y0, y1]).astype(np.float32)


# revision 17
# speedup vs baseline: 1.0433x; 1.0220x over previous
"""Multi-head attention (B=2, N=2048, C=512, H=8) on 8 trn2 NeuronCores.

Sharding: tensor-parallel over heads x data-parallel over batch.
Core i handles batch b = i//4 and heads {2*(i%4), 2*(i%4)+1} (a contiguous
128-column slice of Wq/Wk/Wv and 128-row slice of Wo). Each core computes
its heads' full attention and a partial output projection; the host sums
the 4 partials per batch (the TP row-split unshard) and stacks batches.

v3 design (vs ~200us fp32r v1, ~190us f16 v2):
  - Host pre-transposes q/kv/pos to channel-major and pre-casts all matmul
    operands to fp16 (no on-device input transposes, half the DMA bytes).
  - fp16 matmul operands everywhere, fp32 PSUM accumulation.
  - exp is split across TWO engines per k-chunk: by (kc+h) parity one
    head's [128,1024] S tile goes through ACT's table EXP (~1.11us) and
    the other through a DVE Schraudolph exp (~1.22us): one tensor_scalar
    computing int16(s*SCALE*2^10/ln2 + (15<<10) - magic), whose bit
    pattern IS fp16 exp to 4% max element error; bitcast to f16 feeds PV.
    Softmax renormalization (ones-column denominators summed from the
    same approximate values) cancels most of it: measured end-to-end
    max-rel error ~0.9e-2 in numpy simulation incl. the f16 pipeline.
  - PSUM: 2 single-buffered S tiles [128,1024] + 2 PV accumulators
    [128,1024] = 8 banks. exp of head h chases S bank-pair by bank-pair.
  - PV lhsT is padded to 128 columns ([d(64)|ones|pad] stride-72 layout)
    to enable the fast-weight-load path; the garbage output rows 65-127
    of the PV accumulator are never read.
  - PV issue lags S/exp by 3 k-chunks so the PE queue never blocks on the
    q-half boundary (PV(qh2=1,kc0) waits for the previous half's
    normalize to release the accumulator banks).
  - Input DMAs ride the two HWDGE queues interleaved with the weight/pos
    loads (v2 put 1.6MB on the slow SWDGE path, gating phase 1).
"""
import numpy as np

B, N, C, H, D = 2, 2048, 512, 8, 64
SCALE = float(C) ** -0.5
NCORES = 8
P = 128
CC = C // P            # 4 channel chunks of 128
NT16 = N // P          # 16 token tiles of 128
NT4 = N // 512         # 4 token slabs of 512
PVLAG = 5              # PV issue lag in k-chunks

# Schraudolph fp16 exp constants (folding the attention scale):
#   i16 = rni(s * SCH_A + SCH_B); bitcast f16 ~= exp(s * SCALE)
SCH_A = SCALE * (2.0 ** 10) / float(np.log(2.0))
SCH_B = 15.0 * 1024.0 - 486411.0 / 8192.0

_cached_nc = None


def _build():
    from contextlib import ExitStack

    import concourse.mybir as mybir
    import concourse.tile as tile
    from concourse import bacc
    from concourse.alu_op_type import AluOpType
    from concourse.tile_rust import add_dep_helper

    f32 = mybir.dt.float32
    f16 = mybir.dt.float16
    i16 = mybir.dt.int16
    AF = mybir.ActivationFunctionType

    nc = bacc.Bacc("TRN2", target_bir_lowering=False, debug=False)

    # host-prepared, channel-major, fp16
    qT = nc.dram_tensor("qT", [P, CC, N], f16, kind="ExternalInput")
    kvT = nc.dram_tensor("kvT", [P, CC, N], f16, kind="ExternalInput")
    posq = nc.dram_tensor("posq", [P, N], f16, kind="ExternalInput")  # 2x stacked
    posk = nc.dram_tensor("posk", [P, N], f16, kind="ExternalInput")
    wq = nc.dram_tensor("wq", [P, CC, P], f16, kind="ExternalInput")
    wk = nc.dram_tensor("wk", [P, CC, P], f16, kind="ExternalInput")
    wv = nc.dram_tensor("wv", [P, CC, P], f16, kind="ExternalInput")
    wo = nc.dram_tensor("wo", [P, C], f16, kind="ExternalInput")
    eye = nc.dram_tensor("eye", [P, P], f16, kind="ExternalInput")
    y = nc.dram_tensor("y", [N, C], f32, kind="ExternalOutput")

    with tile.TileContext(nc) as tc, ExitStack() as ctx:
        persist = ctx.enter_context(tc.tile_pool(name="persist", bufs=1))

        # ---- input DMAs on the two HWDGE queues, weights-first ----
        qT_sb = persist.tile([P, CC, N], f16, tag="qT_sb")
        kvT_sb = persist.tile([P, CC, N], f16, tag="kvT_sb")
        w_sb = {name: persist.tile([P, CC, P], f16, tag=f"{name}_sb",
                                   name=f"{name}_sb")
                for name in ("wq", "wk", "wv")}
        pos_sb = {name: persist.tile([P, N], f16, tag=f"pos_{name}",
                                     name=f"pos_{name}")
                  for name in ("q", "k")}
        eye_sb = persist.tile([P, P], f16, tag="eye_sb")

        nc.sync.dma_start(w_sb["wq"][:], wq[:])
        nc.scalar.dma_start(w_sb["wk"][:], wk[:])
        nc.scalar.dma_start(w_sb["wv"][:], wv[:])
        nc.scalar.dma_start(eye_sb[:], eye[:])
        for ntt in range(NT4):
            sl = slice(ntt * 512, (ntt + 1) * 512)
            nc.sync.dma_start(qT_sb[:, :, sl], qT[:, :, sl])
            nc.scalar.dma_start(kvT_sb[:, :, sl], kvT[:, :, sl])
            if ntt == 0:
                nc.sync.dma_start(pos_sb["q"][:], posq[:])
                nc.scalar.dma_start(pos_sb["k"][:], posk[:])
        # phase-3-only loads on the swdge path (bias is added on the host
        # after summing the TP partials: y = sum(partials) + bo)
        wo_r = []
        for h in (0, 1):
            wr = persist.tile([64, C], f16, tag=f"wo_r{h}")
            nc.gpsimd.dma_start(wr[:], wo[64 * h:64 * (h + 1), :])
            wo_r.append(wr)

        # PE order pinned with order-only deps: the PE queue is in-order,
        # so this fixes the matmul stream layout.
        pe_prev = [None]

        def chain(mm):
            if pe_prev[0] is not None:
                add_dep_helper(mm.ins, pe_prev[0].ins, sync=False,
                               reason="pin PE order")
            pe_prev[0] = mm

        # ---- phase 1: projections (channel-major out), vp token-major ----
        qhT = persist.tile([P, N], f16, tag="qhT")
        khT = persist.tile([P, N], f16, tag="khT")
        # vp: flat padded layout, per kc at offset 144*kc:
        #   [h0 d0..63 | ones | pad7 | h1 d0..63 | ones | pad7]
        # lhsT(kc,h) = vp[:, 144*kc+72*h : +128] -> 128 cols (FWL-eligible),
        # cols past 65 are finite garbage feeding unread PV output rows.
        vp = persist.tile([P, NT16 * 144 + 56], f16, tag="vp")
        vp4 = vp[:, 0:NT16 * 144].rearrange("p (a b c) -> p a b c", a=NT16, b=2)
        fill_col = persist.tile([P, 1], f16, tag="fill_col")
        nc.gpsimd.memset(fill_col[:], 0.0)
        nc.vector.tensor_copy(vp4[:, :, :, 65:72],
                              fill_col[:].to_broadcast((P, NT16, 2, 7)))
        nc.vector.tensor_copy(vp[:, NT16 * 144:],
                              fill_col[:].to_broadcast((P, 56)))
        ones_col = persist.tile([P, 1], f16, tag="ones_col")
        nc.gpsimd.memset(ones_col[:], 1.0)
        nc.vector.tensor_copy(vp4[:, :, :, 64:65],
                              ones_col[:].to_broadcast((P, NT16, 2, 1)))

        with (
            tc.tile_pool(name="proj_ps", bufs=3, space="PSUM") as proj_ps,
            tc.tile_pool(name="tp_ps", bufs=4, space="PSUM") as tp_ps,
            tc.tile_pool(name="big", bufs=1) as big,
        ):
            vpT = big.tile([P, N], f16, tag="vpT")
            for ntt in range(NT4):
                sl = slice(ntt * 512, (ntt + 1) * 512)
                for wname, srcT, dstT, posn in (
                    ("wq", qT_sb, qhT, "q"),
                    ("wk", kvT_sb, khT, "k"),
                ):
                    pp = proj_ps.tile([P, 512], f32, tag="proj")
                    for cc in range(CC):
                        chain(nc.tensor.matmul(
                            pp[:], w_sb[wname][:, cc], srcT[:, cc, sl],
                            start=(cc == 0), stop=(cc == CC - 1)))
                    nc.vector.tensor_add(
                        out=dstT[:, sl], in0=pp[:], in1=pos_sb[posn][:, sl])
                pp = proj_ps.tile([P, 512], f32, tag="proj")
                for cc in range(CC):
                    chain(nc.tensor.matmul(
                        pp[:], w_sb["wv"][:, cc], kvT_sb[:, cc, sl],
                        start=(cc == 0), stop=(cc == CC - 1)))
                nc.scalar.copy(vpT[:, sl], pp[:])
                # transpose this slab's 4 token tiles into vp
                for t in range(ntt * 4, ntt * 4 + 4):
                    tp = tp_ps.tile([P, P], f16, tag="tp")
                    chain(nc.tensor.matmul(
                        tp[:], vpT[:, t * P:(t + 1) * P], eye_sb[:],
                        is_transpose=True))
                    nc.vector.tensor_copy(
                        vp4[:, t, :, 0:64],
                        tp[:].rearrange("p (h d) -> p h d", h=2))

        # ---- phase 2: attention ----
        O_sb = persist.tile([64, 2, N], f16, tag="O_sb")
        with (
            tc.tile_pool(name="st_ps", bufs=2, space="PSUM") as st_ps,
            tc.tile_pool(name="ot_ps", bufs=1, space="PSUM") as ot_ps,
            tc.tile_pool(name="expp", bufs=8) as expp,
            tc.tile_pool(name="den", bufs=2) as den_pool,
            tc.tile_pool(name="dend", bufs=2, space="DRAM") as den_dram_pool,
        ):
            for qh2 in range(2):
                q_sl = slice(qh2 * 1024, (qh2 + 1) * 1024)
                OT = [ot_ps.tile([P, 1024], f32, tag=f"ot{h}",
                                 name=f"ot{h}") for h in (0, 1)]

                def s_pair(kc, h):
                    # two [128,512] S tiles (1 PSUM bank each, bufs=2 per
                    # head-lineage) so next kc's S never waits on this exp
                    ex = expp.tile([P, 1024], f16, tag=f"ex{h}",
                                   name=f"ex{h}")
                    hsl = slice(64 * h, 64 * h + 64)
                    for qq in range(2):
                        st = st_ps.tile([P, 512], f32, tag=f"st{h}",
                                        name=f"st{h}q{qq}")
                        chain(nc.tensor.matmul(
                            st[:],
                            khT[hsl, kc * P:(kc + 1) * P],
                            qhT[hsl, (qh2 * 2 + qq) * 512:
                                     (qh2 * 2 + qq + 1) * 512],
                            start=True, stop=True))
                        exq_sl = ex[:, qq * 512:(qq + 1) * 512]
                        if (kc + h) % 2 == 0:
                            nc.scalar.activation(exq_sl, st[:], AF.Exp,
                                                 scale=SCALE)
                        else:
                            nc.vector.tensor_scalar(
                                exq_sl.bitcast(i16), st[:], SCH_A, SCH_B,
                                AluOpType.mult, AluOpType.add)
                    return ex

                def pv_quad(kc, exs):
                    for h in (0, 1):
                        lhsT = vp[:, 144 * kc + 72 * h:144 * kc + 72 * h + 128]
                        for qq in range(2):
                            chain(nc.tensor.matmul(
                                OT[h][:, qq * 512:(qq + 1) * 512],
                                lhsT,
                                exs[h][:, qq * 512:(qq + 1) * 512],
                                start=(kc == 0), stop=(kc == NT16 - 1)))

                exq = []
                for kc in range(NT16):
                    exq.append([s_pair(kc, 0), s_pair(kc, 1)])
                    if kc >= PVLAG:
                        pv_quad(kc - PVLAG, exq[kc - PVLAG])
                for kc in range(NT16 - PVLAG, NT16):
                    pv_quad(kc, exq[kc])

                # normalize: O = O' / den; den-row copies on ACT, recip +
                # multiplies on DVE, broadcast via DRAM bounce.
                den_r = den_pool.tile([P, 2, 1024], f32, tag="den_r")
                for h in (0, 1):
                    nc.scalar.copy(den_r[64:65, h, :], OT[h][64:65, :])
                den_d = den_dram_pool.tile([1, 2, 1024], f32, tag="den_d")
                nc.sync.dma_start(den_d[:], den_r[64:65, :, :])
                den_bc = den_pool.tile([64, 2, 1024], f32, tag="den_bc")
                nc.sync.dma_start(
                    den_bc[:], den_d[:].to_broadcast((64, 2, 1024)))
                den_rec = den_pool.tile([64, 2, 1024], f32, tag="den_rec")
                nc.vector.reciprocal_approx_fast(den_rec[:], den_bc[:])
                for h in (0, 1):
                    nc.vector.tensor_mul(
                        out=O_sb[:, h, q_sl], in0=OT[h][0:64, :],
                        in1=den_rec[:, h, :])

        # ---- phase 3: output projection, token-major ----
        with (
            tc.tile_pool(name="y_ps", bufs=2, space="PSUM") as y_ps,
            tc.tile_pool(name="yout", bufs=3) as yout,
        ):
            for t in range(NT16):
                tsl = slice(t * P, (t + 1) * P)
                yp = y_ps.tile([P, C], f32, tag="y")
                for h in (0, 1):
                    chain(nc.tensor.matmul(
                        yp[:], O_sb[:, h, tsl], wo_r[h][:],
                        start=(h == 0), stop=(h == 1)))
                ysb = yout.tile([P, C], f32, tag="ysb")
                if t % 2 == 0:
                    nc.scalar.copy(ysb[:], yp[:])
                    nc.sync.dma_start(y[tsl, :], ysb[:])
                else:
                    nc.vector.tensor_copy(ysb[:], yp[:])
                    nc.scalar.dma_start(y[tsl, :], ysb[:])

    nc.finalize()
    return nc


def _chmajor(x):
    # [N, C] token-major f32 -> [P, CC, N] channel-major chunked f16
    return np.ascontiguousarray(
        x.T.reshape(CC, P, N).transpose(1, 0, 2)).astype(np.float16)


def _wchunk(w):
    # [C, P] -> [P, CC, P] lhsT chunks
    return np.ascontiguousarray(
        w.reshape(CC, P, P).transpose(1, 0, 2)).astype(np.float16)


def _in_maps(q, kv, pos_q, pos_k, Wq, Wk, Wv, Wo, bo):
    maps = []
    for i in range(NCORES):
        b, hp = i // 4, i % 4
        cs = P * hp
        posqT = pos_q[b].T.astype(np.float16)   # [64, N]
        poskT = pos_k[b].T.astype(np.float16)
        maps.append({
            "qT": _chmajor(np.asarray(q[b], dtype=np.float32)),
            "kvT": _chmajor(np.asarray(kv[b], dtype=np.float32)),
            "posq": np.ascontiguousarray(np.concatenate([posqT, posqT], 0)),
            "posk": np.ascontiguousarray(np.concatenate([poskT, poskT], 0)),
            "wq": _wchunk(np.asarray(Wq[:, cs:cs + P], dtype=np.float32)),
            "wk": _wchunk(np.asarray(Wk[:, cs:cs + P], dtype=np.float32)),
            "wv": _wchunk(np.asarray(Wv[:, cs:cs + P], dtype=np.float32)),
            "wo": np.ascontiguousarray(Wo[cs:cs + P, :]).astype(np.float16),
            "eye": np.eye(P, dtype=np.float16),
        })
    return maps


def kernel(q, kv, pos_q, pos_k, Wq, Wk, Wv, Wo, bo):
    from concourse.bass_utils import run_bass_kernel_spmd

    global _cached_nc
    if _cached_nc is None:
        _cached_nc = _build()

    args = [np.asarray(a) for a in (q, kv, pos_q, pos_k, Wq, Wk, Wv, Wo, bo)]
    maps = _in_maps(*args)
    res = run_bass_kernel_spmd(_cached_nc, maps, list(range(NCORES)))
    outs = [res.results[i]["y"] for i in range(NCORES)]
    bo32 = np.asarray(args[8], dtype=np.float32)
    y0 = outs[0] + outs[1] + outs[2] + outs[3] + bo32
    y1 = outs[4] + outs[5] + outs[6] + outs[7] + bo32
    return np.stack([y0, y1]).astype(np.float32)
